# revision 22
# baseline (speedup 1.0000x reference)
"""Trainium2 Bass kernel for ChannelHyperedgeInteraction.

Computation (per batch b):
    E  = masked-mean-pool of x over L              [C, d]
    qkv = E @ Wqkv + bqkv ; q,k,v                  [C, d] each
    S  = (q k^T / sqrt(d)) * (0.5 + 0.5*overlap)   [C, C]
    A  = softmax(S, -1) ; out = A v @ Wo + bo
    E  = LN(E + out) ; h = gelu(E W1 + b1) W2 + b2 ; return LN(E + h)

Sharding: data-parallel over B across the 8 NeuronCores (2 batches/core).
Weights are replicated. Each core computes its own y[b_local] slice; the
host concatenates.

Key kernel design points:
 - The dominant cost is streaming x (134 MB/core). The masked pooling is
   done on the TensorEngine as E += diag(mask_scaled[:, l]) @ x[b, :, l, :]
   accumulated over all 256 l into one PSUM bank; the diagonal stationary
   operands are built on the VectorEngine four-at-a-time from an identity
   block times a broadcast mask column. float32r is used so the moving
   operand streams at 1 column/cycle.
 - The overlap bias term is computed as matmuls of mask^T blocks.
 - All contractions over d use PE-transposed operands (identity-matmul
   transposes); biases are folded into PSUM with K=1 ones-row matmuls.
 - Gelu uses the tanh approximation built from DVE/ACT primitives.
"""

import math
from contextlib import ExitStack

import numpy as np

import concourse.bass as bass
import concourse.mybir as mybir
import concourse.tile as tile
from concourse.bass_utils import run_bass_kernel_spmd
from concourse.masks import make_identity

F32 = mybir.dt.float32
F32R = mybir.dt.float32r
AX = mybir.AxisListType
OP = mybir.AluOpType
ACTF = mybir.ActivationFunctionType

B, C, L, D = 16, 128, 256, 512
N_CORES = 8
B_LOCAL = B // N_CORES  # 2
P = 128
LC = L // P   # 2 l-chunks
DC = D // P   # 4 d-chunks
D2C = (2 * D) // P  # 8 chunks of the FFN hidden dim
NLB = 4  # l-positions per x DMA / diag batch (1 MB transfers)

WEIGHT_NAMES = ("Wqkv", "bqkv", "Wo", "bo", "W1", "b1", "W2", "b2",
                "g1", "be1", "g2", "be2")


def _build_kernel_body(ctx: ExitStack, tc: "tile.TileContext", io: dict):
    nc = tc.nc
    x, mask, y = io["x"], io["mask"], io["y"]

    singles = ctx.enter_context(tc.tile_pool(name="singles", bufs=1))
    xpool = ctx.enter_context(tc.tile_pool(name="xpool", bufs=3))
    work = ctx.enter_context(tc.tile_pool(name="work", bufs=2))
    big = ctx.enter_context(tc.tile_pool(name="big", bufs=1))
    psE = ctx.enter_context(tc.tile_pool(name="psE", bufs=1, space="PSUM"))
    psT = ctx.enter_context(tc.tile_pool(name="psT", bufs=2, space="PSUM"))
    psMM = ctx.enter_context(tc.tile_pool(name="psMM", bufs=3, space="PSUM"))
    psD = ctx.enter_context(tc.tile_pool(name="psD", bufs=1, space="PSUM"))

    ones_row = singles.tile([1, P], F32)
    nc.vector.memset(ones_row, 1.0)
    eps_t = singles.tile([P, 1], F32)
    nc.vector.memset(eps_t, 1e-5)
    # identity blocks: template for the pooling diag weights
    identN = singles.tile([P, NLB, P], F32)
    nc.gpsimd.memset(identN, 0.0)
    for i in range(NLB):
        make_identity(nc, identN[:, i, :], nomemset=True)
    # fp32 identity for PE transposes. Created LAST of the gpsimd constants:
    # the warmup transpose below makes PE observe the gpsimd semaphore at
    # this (latest) tick, so no later PE instruction needs a Pool wait —
    # keeps every self-loading matmul at <=1 sync wait (S3_LW ISA limit).
    ident = singles.tile([P, P], F32)
    make_identity(nc, ident)
    wu_ps = psT.tile([P, P], F32, tag="pst")
    nc.tensor.transpose(wu_ps, ident, ident)

    # --- weights / per-d vectors, loaded once ------------------------------
    # (issued lazily below, after the first batch's pooling DMAs, so the x
    # stream is not stuck behind 8 MB of weights at kernel start)
    wstate = {}

    def load_weights():
        w = {}
        w["wqkv"] = singles.tile([P, DC, 3 * D], F32, name="wqkv_sb")
        nc.sync.dma_start(out=w["wqkv"],
                          in_=io["Wqkv"][:].rearrange("(j p) n -> p j n", p=P))
        w["w1"] = singles.tile([P, DC, 2 * D], F32, name="w1_sb")
        nc.sync.dma_start(out=w["w1"],
                          in_=io["W1"][:].rearrange("(j p) n -> p j n", p=P))
        w["w2"] = singles.tile([P, D2C, D], F32, name="w2_sb")
        nc.sync.dma_start(out=w["w2"],
                          in_=io["W2"][:].rearrange("(j p) n -> p j n", p=P))
        w["wo"] = singles.tile([P, DC, D], F32, name="wo_sb")
        nc.sync.dma_start(out=w["wo"],
                          in_=io["Wo"][:].rearrange("(j p) n -> p j n", p=P))
        # biases as single-partition rows (added into PSUM via K=1 matmuls)
        for nm, width in (("bqkv", 3 * D), ("bo", D), ("b1", 2 * D), ("b2", D)):
            t = singles.tile([1, width], F32, name=f"row_{nm}")
            nc.sync.dma_start(out=t, in_=io[nm][None, :])
            w[nm] = t
        # LN affine vectors broadcast across partitions
        for nm in ("g1", "be1", "g2", "be2"):
            t = singles.tile([P, D], F32, name=f"bc_{nm}")
            nc.gpsimd.dma_start(
                out=t, in_=io[nm][None, :].to_broadcast((P, D)))
            w[nm] = t
        return w

    # One kernel-long accumulation group of 1x1 "gate" matmuls. Each gate
    # reads one element of a freshly DMA-ed tile so PE observes that DMA's
    # completion semaphore on a cheap instruction (1 wait) instead of on a
    # real matmul (which already carries its one allowed sync wait). A single
    # group avoids PE self-waits from PSUM slot WAW between groups.
    N_GATES = 2 * (L // NLB) + 8
    _gate = {"ps": None, "i": 0}

    def gate_mm(el):
        if _gate["ps"] is None:
            _gate["ps"] = psD.tile([1, 1], F32, tag="gate", name="gate_ps")
        nc.tensor.matmul(_gate["ps"], lhsT=el, rhs=el,
                         start=(_gate["i"] == 0),
                         stop=(_gate["i"] == N_GATES - 1))
        _gate["i"] += 1

    def observe_weight_dmas(w):
        for nm in ("wqkv", "w1", "w2", "wo", "bqkv", "bo", "b1", "b2"):
            t = w[nm]
            el = t[0:1, 0:1] if len(t.shape) == 2 else t[0:1, 0, 0:1]
            gate_mm(el)

    def bias_mm(psum_ap, bias_row_ap):
        """Final accumulation-group matmul adding a [1, N] bias row to all
        output rows: psum += ones[K=1, M=P].T @ bias[K=1, N]."""
        nc.tensor.matmul(psum_ap, lhsT=ones_row,
                         rhs=bias_row_ap, start=False, stop=True)

    def transpose_chunks(src, nch, tag):
        """[P, nch*P] SBUF -> [P, nch, P] SBUF holding src^T chunks."""
        dst = work.tile([P, nch, P], F32, tag=tag)
        for j in range(nch):
            ps = psT.tile([P, P], F32, tag="pst")
            nc.tensor.transpose(ps, src[:, j * P:(j + 1) * P], ident)
            nc.vector.tensor_copy(dst[:, j], ps)
        return dst

    def layernorm(src, g_b, be_b, tag):
        stats = work.tile([P, 6], F32, tag=tag + "_st")
        nc.vector.bn_stats(out=stats, in_=src)
        mv = work.tile([P, 2], F32, tag=tag + "_mv")
        nc.vector.bn_aggr(out=mv, in_=stats)
        rstd = work.tile([P, 1], F32, tag=tag + "_rs")
        nc.scalar.activation(rstd, mv[:, 1:2], ACTF.Sqrt, bias=eps_t)
        nc.vector.reciprocal(rstd, rstd)
        out_t = work.tile([P, D], F32, tag=tag)
        nc.vector.tensor_scalar(out_t, src, scalar1=mv[:, 0:1], scalar2=rstd,
                                op0=OP.subtract, op1=OP.mult)
        nc.vector.tensor_mul(out_t, out_t, g_b)
        nc.vector.tensor_add(out_t, out_t, be_b)
        return out_t

    def mask_prep(b):
        """Returns (ms_t [P,L] row-normalized mask, factor [P,P])."""
        mb_t = work.tile([P, L], F32, tag="mb")
        nc.sync.dma_start(out=mb_t, in_=mask[b])
        total = work.tile([P, 1], F32, tag="total")
        nc.vector.reduce_sum(total, mb_t, axis=AX.X, op=OP.add)
        rden = work.tile([P, 1], F32, tag="rden")
        nc.vector.tensor_scalar_max(rden, total, 1.0)
        nc.vector.reciprocal(rden, rden)
        ms_t = work.tile([P, L], F32, tag="ms")
        nc.vector.tensor_scalar_mul(ms_t, mb_t, rden)

        mT = transpose_chunks(mb_t, LC, "mT")      # raw mask^T  [l, c]

        # joint[c,e] = sum_l m[c,l] m[e,l]
        joint_ps = psMM.tile([P, P], F32, tag="mm")
        for ch in range(LC):
            nc.tensor.matmul(joint_ps, lhsT=mT[:, ch], rhs=mT[:, ch],
                             start=(ch == 0), stop=(ch == LC - 1))
        # broadcast total^T along rows
        totT_ps = psT.tile([1, P], F32, tag="pst")
        nc.tensor.transpose(totT_ps, total, ident)
        tot_row = work.tile([1, P], F32, tag="totrow")
        nc.vector.tensor_copy(tot_row, totT_ps)
        totb_ps = psT.tile([P, P], F32, tag="pst")
        nc.tensor.matmul(totb_ps, lhsT=ones_row, rhs=tot_row,
                         start=True, stop=True)
        # factor = (0.5 + joint / max(total[c]+total[e], 1)) / sqrt(D)
        factor = work.tile([P, P], F32, tag="factor")
        nc.vector.tensor_scalar_add(factor, totb_ps, total)
        nc.vector.tensor_scalar_max(factor, factor, 1.0)
        nc.vector.reciprocal(factor, factor)
        nc.vector.tensor_mul(factor, factor, joint_ps)
        nc.vector.tensor_scalar(factor, factor, 0.5, 1.0 / math.sqrt(D),
                                op0=OP.add, op1=OP.mult)
        return ms_t, factor

    def pooling(b, ms_t):
        """Masked-mean pooling -> E_sb [P(c), D].

        E += diag(ms[:, l]) @ x[b, :, l, :], accumulated over l in PSUM.
        """
        psum_E = psE.tile([P, D], F32, tag="psE")
        for ib, l0 in enumerate(range(0, L, NLB)):
            xt = xpool.tile([P, NLB, D], F32R, tag="xt")
            nc.sync.dma_start(out=xt, in_=x[b, :, l0:l0 + NLB, :].bitcast(F32R))
            diag = work.tile([P, NLB, P], F32R, tag="diag")
            nc.vector.tensor_tensor(
                diag, identN,
                ms_t[:, l0:l0 + NLB, None].to_broadcast((P, NLB, P)),
                OP.mult)
            gate_mm(xt[0:1, 0, 0:1].bitcast(F32))
            for i in range(NLB):
                nc.tensor.matmul(
                    psum_E,
                    lhsT=diag[:, i],
                    rhs=xt[:, i],
                    start=(l0 == 0 and i == 0),
                    stop=(l0 == L - NLB and i == NLB - 1),
                )
        E_sb = work.tile([P, D], F32, tag="E")
        nc.vector.tensor_copy(E_sb, psum_E)
        return E_sb

    def transformer(b, E_sb, factor, w):
        ET = transpose_chunks(E_sb, DC, "ET")

        # qkv projection (contract over d), bias folded in via K=1 matmul
        qkv_ps = [psMM.tile([P, D], F32, tag="mm", name=f"qkv_ps{i}") for i in range(3)]
        for i in range(3):
            for j in range(DC):
                nc.tensor.matmul(qkv_ps[i], lhsT=ET[:, j],
                                 rhs=w["wqkv"][:, j, i * D:(i + 1) * D],
                                 start=(j == 0), stop=False)
            bias_mm(qkv_ps[i], w["bqkv"][:, i * D:(i + 1) * D])
        q_sb = work.tile([P, D], F32, tag="q")
        nc.vector.tensor_copy(q_sb, qkv_ps[0])
        k_sb = work.tile([P, D], F32, tag="k")
        nc.vector.tensor_copy(k_sb, qkv_ps[1])
        v_sb = work.tile([P, D], F32, tag="v")
        nc.vector.tensor_copy(v_sb, qkv_ps[2])

        # S = q k^T, then bias/scale, softmax
        qT = transpose_chunks(q_sb, DC, "qT")
        kT = transpose_chunks(k_sb, DC, "kT")
        S_ps = psMM.tile([P, P], F32, tag="mm")
        for j in range(DC):
            nc.tensor.matmul(S_ps, lhsT=qT[:, j], rhs=kT[:, j],
                             start=(j == 0), stop=(j == DC - 1))
        Sb = work.tile([P, P], F32, tag="Sb")
        nc.vector.tensor_mul(Sb, S_ps, factor)
        nmax = work.tile([P, 1], F32, tag="nmax")
        nc.vector.reduce_max(nmax, Sb, axis=AX.X, negate=True)
        Pexp = work.tile([P, P], F32, tag="Pexp")
        sumexp = work.tile([P, 1], F32, tag="sumexp")
        nc.scalar.activation(Pexp, Sb, ACTF.Exp, bias=nmax, scale=1.0,
                             accum_out=sumexp)
        rinv = work.tile([P, 1], F32, tag="rinv")
        nc.vector.reciprocal(rinv, sumexp)
        # normalize rows of exp(S) so attn = A @ v directly
        nc.vector.tensor_scalar_mul(Pexp, Pexp, rinv)
        PT_ps = psT.tile([P, P], F32, tag="pst")
        nc.tensor.transpose(PT_ps, Pexp, ident)
        PT_sb = work.tile([P, P], F32, tag="PT")
        nc.vector.tensor_copy(PT_sb, PT_ps)
        attn_ps = psMM.tile([P, D], F32, tag="mm")
        nc.tensor.matmul(attn_ps, lhsT=PT_sb, rhs=v_sb, start=True, stop=True)
        attn_sb = work.tile([P, D], F32, tag="attnsb")
        nc.vector.tensor_copy(attn_sb, attn_ps)

        # o = attn @ Wo + bo ; res1 = E + o ; E1 = LN1(res1)
        attnT = transpose_chunks(attn_sb, DC, "attnT")
        o_ps = psMM.tile([P, D], F32, tag="mm")
        for j in range(DC):
            nc.tensor.matmul(o_ps, lhsT=attnT[:, j], rhs=w["wo"][:, j],
                             start=(j == 0), stop=False)
        bias_mm(o_ps, w["bo"])
        res1 = work.tile([P, D], F32, tag="res1")
        nc.vector.tensor_add(res1, o_ps, E_sb)
        E1 = layernorm(res1, w["g1"], w["be1"], "E1")

        # FFN: h = gelu_tanh(E1 @ W1 + b1) ; o2 = h @ W2 + b2
        E1T = transpose_chunks(E1, DC, "E1T")
        h_ps = [psMM.tile([P, D], F32, tag="mm", name=f"h_ps{i}") for i in range(2)]
        for i in range(2):
            for j in range(DC):
                nc.tensor.matmul(h_ps[i], lhsT=E1T[:, j],
                                 rhs=w["w1"][:, j, i * D:(i + 1) * D],
                                 start=(j == 0), stop=False)
            bias_mm(h_ps[i], w["b1"][:, i * D:(i + 1) * D])
        hpre = big.tile([P, 2 * D], F32, tag="hpre")
        nc.vector.tensor_copy(hpre[:, 0:D], h_ps[0])
        nc.vector.tensor_copy(hpre[:, D:2 * D], h_ps[1])
        t1 = big.tile([P, 2 * D], F32, tag="t1")
        nc.vector.tensor_mul(t1, hpre, hpre)
        nc.vector.tensor_scalar(t1, t1, 0.044715, 1.0, op0=OP.mult, op1=OP.add)
        nc.vector.tensor_mul(t1, t1, hpre)
        nc.scalar.activation(t1, t1, ACTF.Tanh,
                             scale=math.sqrt(2.0 / math.pi))
        nc.vector.tensor_scalar(t1, t1, 1.0, 0.5, op0=OP.add, op1=OP.mult)
        nc.vector.tensor_mul(t1, t1, hpre)  # t1 = gelu(hpre)
        hT = transpose_chunks_big(t1, D2C, "hT")
        o2_ps = psMM.tile([P, D], F32, tag="mm")
        for jj in range(D2C):
            nc.tensor.matmul(o2_ps, lhsT=hT[:, jj], rhs=w["w2"][:, jj],
                             start=(jj == 0), stop=False)
        bias_mm(o2_ps, w["b2"])
        res2 = work.tile([P, D], F32, tag="res2")
        nc.vector.tensor_add(res2, o2_ps, E1)
        y_sb = layernorm(res2, w["g2"], w["be2"], "yln")
        nc.sync.dma_start(out=y[b], in_=y_sb)

    def transpose_chunks_big(src, nch, tag):
        dst = big.tile([P, nch, P], F32, tag=tag)
        for j in range(nch):
            ps = psT.tile([P, P], F32, tag="pst")
            nc.tensor.transpose(ps, src[:, j * P:(j + 1) * P], ident)
            nc.vector.tensor_copy(dst[:, j], ps)
        return dst

    # ---- main schedule ----------------------------------------------------
    wstate.update(load_weights())
    for b in range(B_LOCAL):
        ms_t, factor = mask_prep(b)
        E_sb = pooling(b, ms_t)
        if b == 0:
            observe_weight_dmas(wstate)
        transformer(b, E_sb, factor, wstate)


def build_module() -> bass.Bass:
    # Funnel all HWDGE DMA completions onto one semaphore lane. nc.sync DMAs
    # issue FIFO from the SP ring and complete in order, so cumulative waits
    # on a single lane are safe — and every DMA-dependent instruction then
    # needs at most ONE DMA semaphore wait, which keeps us inside the
    # per-instruction sync-wait limits of the walrus codegen structs.
    import concourse.tile_sem_assignment as _tsa
    _tsa.NUM_HWDGE_SEMS = 1

    nc = bass.Bass()
    io = {}
    io["x"] = nc.declare_dram_parameter("x", [B_LOCAL, C, L, D], F32,
                                        isOutput=False)
    io["mask"] = nc.declare_dram_parameter("mask", [B_LOCAL, C, L], F32,
                                           isOutput=False)
    shapes = {
        "Wqkv": [D, 3 * D], "bqkv": [3 * D], "Wo": [D, D], "bo": [D],
        "W1": [D, 2 * D], "b1": [2 * D], "W2": [2 * D, D], "b2": [D],
        "g1": [D], "be1": [D], "g2": [D], "be2": [D],
    }
    for nm in WEIGHT_NAMES:
        io[nm] = nc.declare_dram_parameter(nm, shapes[nm], F32, isOutput=False)
    io["y"] = nc.declare_dram_parameter("y", [B_LOCAL, C, D], F32,
                                        isOutput=True)

    with tile.TileContext(nc) as tc:
        with ExitStack() as ctx:
            _build_kernel_body(ctx, tc, io)
    _split_multi_waits(nc)
    return nc


def _split_multi_waits(nc: bass.Bass) -> int:
    """The walrus codegen in this toolchain accepts at most ONE sync-wait
    command per ISA instruction. Tile's semaphore assignment can attach
    several. Spill all but the last wait of each instruction onto NoOp
    instructions (same engine, inserted just before it), each carrying a
    single wait — execution-equivalent since the engine stream is in-order.
    """
    import bass_rust as _br
    fn = nc.m.functions[0]
    n_spilled = 0
    for blk in fn.blocks:
        out = []
        changed = False
        for inst in blk.instructions:
            si = inst.sync_info
            if si is not None and len(si.on_wait) > 1:
                waits = list(si.on_wait)
                for w in waits[:-1]:
                    n_spilled += 1
                    nop = mybir.InstNoOp(
                        name=f"I-wspill-{n_spilled}",
                        engine=inst.engine,
                        sync_info=_br.SyncInfo(on_wait=[w], on_update=[]),
                        bass_nofuse=True,
                    )
                    nc.register_instruction(nop)
                    out.append(nop)
                inst.sync_info = _br.SyncInfo(
                    on_wait=[waits[-1]], on_update=list(si.on_update))
                changed = True
            out.append(inst)
        if changed:
            blk.instructions = out
    return n_spilled


_NC_CACHE = None


def _get_module():
    global _NC_CACHE
    if _NC_CACHE is None:
        _NC_CACHE = build_module()
    return _NC_CACHE


def kernel(**inputs) -> np.ndarray:
    arrs = {k: np.ascontiguousarray(np.asarray(v, dtype=np.float32))
            for k, v in inputs.items()}
    nc = _get_module()
    in_maps = []
    for i in range(N_CORES):
        m = {
            "x": arrs["x"][i * B_LOCAL:(i + 1) * B_LOCAL],
            "mask": arrs["mask"][i * B_LOCAL:(i + 1) * B_LOCAL],
        }
        for nm in WEIGHT_NAMES:
            m[nm] = arrs[nm]
        in_maps.append(m)
    res = run_bass_kernel_spmd(nc, in_maps, list(range(N_CORES)))
    return np.concatenate([r["y"] for r in res.results], axis=0)


if __name__ == "__main__":
    nc = build_module()
    print("module built OK;",
          sum(len(getattr(e, 'insts', [])) for e in []) or "")


# revision 27
# speedup vs baseline: 37051.3121x; 37051.3121x over previous
"""Trainium2 Bass kernel for ChannelHyperedgeInteraction.

Computation (per batch b):
    E  = masked-mean-pool of x over L              [C, d]
    qkv = E @ Wqkv + bqkv ; q,k,v                  [C, d] each
    S  = (q k^T / sqrt(d)) * (0.5 + 0.5*overlap)   [C, C]
    A  = softmax(S, -1) ; out = A v @ Wo + bo
    E  = LN(E + out) ; h = gelu(E W1 + b1) W2 + b2 ; return LN(E + h)

Sharding: data-parallel over B across the 8 NeuronCores (2 batches/core).
Weights are replicated. Each core computes its own y[b_local] slice; the
host concatenates.

Key kernel design points:
 - The dominant cost is streaming x (134 MB/core). The masked pooling is
   done on the TensorEngine as E += diag(mask_scaled[:, l]) @ x[b, :, l, :]
   accumulated over all 256 l into one PSUM bank; the diagonal stationary
   operands are built on the VectorEngine from an identity block times a
   broadcast mask column. fp32r (single-pass fp32) keeps the moving
   operand at 1 column/cycle; the x pipeline is 6 tiles deep so the PE
   never starves (and HAM stays un-throttled).
 - The walrus codegen here accepts at most ONE sync wait per ISA
   instruction: all HWDGE DMA completions are funneled onto a single
   semaphore lane, 1x1 "gate" matmuls make PE observe DMA semaphores on
   cheap instructions, and a post-pass spills any remaining multi-waits
   onto same-engine NoOps.
 - All contractions over d use PE-transposed operands (identity-matmul
   transposes); biases are folded into PSUM with K=1 ones-row matmuls.
 - Gelu uses the tanh approximation built from DVE/ACT primitives.
"""

import math
from contextlib import ExitStack

import numpy as np

import concourse.bass as bass
import concourse.mybir as mybir
import concourse.tile as tile
from concourse.bass_utils import run_bass_kernel_spmd
from concourse.masks import make_identity

F32 = mybir.dt.float32
F32R = mybir.dt.float32r
AX = mybir.AxisListType
OP = mybir.AluOpType
ACTF = mybir.ActivationFunctionType

B, C, L, D = 16, 128, 256, 512
N_CORES = 8
B_LOCAL = B // N_CORES  # 2
P = 128
LC = L // P   # 2 l-chunks
DC = D // P   # 4 d-chunks
D2C = (2 * D) // P  # 8 chunks of the FFN hidden dim
NLB = 4  # l-positions per x DMA / diag batch (1 MB transfers)
NBLK = L // NLB

WEIGHT_NAMES = ("Wqkv", "bqkv", "Wo", "bo", "W1", "b1", "W2", "b2",
                "g1", "be1", "g2", "be2")


def _build_kernel_body(ctx: ExitStack, tc: "tile.TileContext", io: dict):
    nc = tc.nc
    x, mask, y = io["x"], io["mask"], io["y"]

    singles = ctx.enter_context(tc.tile_pool(name="singles", bufs=1))
    xpool = ctx.enter_context(tc.tile_pool(name="xpool", bufs=7))
    work2 = ctx.enter_context(tc.tile_pool(name="work2", bufs=2))
    work1 = ctx.enter_context(tc.tile_pool(name="work1", bufs=1))
    psE = ctx.enter_context(tc.tile_pool(name="psE", bufs=1, space="PSUM"))
    psT = ctx.enter_context(tc.tile_pool(name="psT", bufs=2, space="PSUM"))
    psMM = ctx.enter_context(tc.tile_pool(name="psMM", bufs=4, space="PSUM"))
    psD = ctx.enter_context(tc.tile_pool(name="psD", bufs=1, space="PSUM"))

    ones_row = singles.tile([1, P], F32)
    nc.vector.memset(ones_row, 1.0)
    eps_t = singles.tile([P, 1], F32)
    nc.vector.memset(eps_t, 1e-5)
    # scratch operand for HAM warm-keeper matmuls (content irrelevant)
    warm_sb = singles.tile([P, D], F32)
    nc.vector.memset(warm_sb, 1.0)
    # identity blocks: template for the pooling diag weights
    identN = singles.tile([P, NLB, P], F32)
    nc.gpsimd.memset(identN, 0.0)
    for i in range(NLB):
        make_identity(nc, identN[:, i, :], nomemset=True)
    # fp32 identity for PE transposes. Created LAST of the gpsimd constants:
    # the warmup transpose below makes PE observe the gpsimd semaphore at
    # its latest tick, so no later PE instruction needs a Pool wait —
    # keeps every self-loading matmul at <=1 sync wait (ISA limit).
    ident = singles.tile([P, P], F32)
    make_identity(nc, ident)
    wu_ps = psT.tile([P, P], F32, tag="pst")
    nc.tensor.transpose(wu_ps, ident, ident)

    # --- weights (fp32r so matmuls run single-pass) ------------------------
    wstate = {}

    def weight_load_steps():
        """List of () -> None closures, one DMA each, so weight loads can be
        interleaved into the early x stream instead of delaying it."""
        steps = []
        w = wstate

        def big_w(key, src_name, nch, width):
            def go():
                t = singles.tile([P, nch, width], F32R, name=f"{key}_sb")
                nc.sync.dma_start(
                    out=t,
                    in_=io[src_name][:].bitcast(F32R).rearrange(
                        "(j p) n -> p j n", p=P))
                w[key] = t
            return go

        steps.append(big_w("wqkv", "Wqkv", DC, 3 * D))
        steps.append(big_w("w1", "W1", DC, 2 * D))
        steps.append(big_w("w2", "W2", D2C, D))
        steps.append(big_w("wo", "Wo", DC, D))

        def bias_row(key, width):
            def go():
                t = singles.tile([1, width], F32, name=f"row_{key}")
                nc.sync.dma_start(out=t, in_=io[key][None, :])
                w[key] = t
            return go

        for nm, width in (("bqkv", 3 * D), ("bo", D), ("b1", 2 * D), ("b2", D)):
            steps.append(bias_row(nm, width))

        def bcast_vec(key):
            def go():
                t = singles.tile([P, D], F32, name=f"bc_{key}")
                nc.gpsimd.dma_start(
                    out=t, in_=io[key][None, :].to_broadcast((P, D)))
                w[key] = t
            return go

        for nm in ("g1", "be1", "g2", "be2"):
            steps.append(bcast_vec(nm))
        return steps

    # One kernel-long accumulation group of 1x1 "gate" matmuls. Each gate
    # reads one element of a freshly DMA-ed tile so PE observes that DMA's
    # completion semaphore on a cheap instruction (1 wait) instead of on a
    # real matmul (which already carries its one allowed sync wait). A single
    # group avoids PE self-waits from PSUM slot WAW between groups.
    N_GATES = B_LOCAL * NBLK + 8
    _gate = {"ps": None, "i": 0}

    def gate_mm(el):
        if _gate["ps"] is None:
            _gate["ps"] = psD.tile([1, 1], F32, tag="gate", name="gate_ps")
        nc.tensor.matmul(_gate["ps"], lhsT=el, rhs=el,
                         start=(_gate["i"] == 0),
                         stop=(_gate["i"] == N_GATES - 1))
        _gate["i"] += 1

    def observe_weight_dmas():
        for nm in ("wqkv", "w1", "w2", "wo"):
            gate_mm(wstate[nm][0:1, 0, 0:1].bitcast(F32))
        for nm in ("bqkv", "bo", "b1", "b2"):
            gate_mm(wstate[nm][0:1, 0:1])

    # HAM warm-keeper: a long-lived accumulation group of N=512 fp32
    # matmuls on scratch data, sprinkled through PE-idle or transpose-heavy
    # stretches (PE transposes don't register as "busy" with the HAM clock
    # monitor, so K drops to 4/8 and everything runs at half clock without
    # these).
    _warm = {"ps": None, "n": 0}

    def warm_tick(k=1):
        if _warm["ps"] is None:
            _warm["ps"] = psMM.tile([P, D], F32, tag="mm", name="warm_ps")
        for _ in range(k):
            nc.tensor.matmul(_warm["ps"], lhsT=warm_sb[:, 0:P], rhs=warm_sb,
                             start=(_warm["n"] == 0), stop=False)
            _warm["n"] += 1

    def warm_finish():
        if _warm["ps"] is not None:
            nc.tensor.matmul(_warm["ps"], lhsT=warm_sb[:, 0:P], rhs=warm_sb,
                             start=False, stop=True)
            _warm["ps"] = None
            _warm["n"] = 0

    def bias_mm(psum_ap, bias_row_ap):
        """Final accumulation-group matmul adding a [1, N] bias row to all
        output rows: psum += ones[K=1, M=P].T @ bias[K=1, N]."""
        nc.tensor.matmul(psum_ap, lhsT=ones_row,
                         rhs=bias_row_ap, start=False, stop=True)

    def transpose_chunks(src, nch, tag, dtype=F32, warm=False):
        """[P, nch*P] SBUF -> [P, nch, P] SBUF holding src^T chunks."""
        dst = work1.tile([P, nch, P], dtype, tag=tag)
        for j in range(nch):
            ps = psT.tile([P, P], F32, tag="pst")
            nc.tensor.transpose(ps, src[:, j * P:(j + 1) * P], ident)
            nc.vector.tensor_copy(dst[:, j], ps)
        return dst

    def layernorm(src, g_b, be_b, tag):
        stats = work1.tile([P, 6], F32, tag=tag + "_st")
        nc.vector.bn_stats(out=stats, in_=src)
        mv = work1.tile([P, 2], F32, tag=tag + "_mv")
        nc.vector.bn_aggr(out=mv, in_=stats)
        rstd = work1.tile([P, 1], F32, tag=tag + "_rs")
        nc.scalar.activation(rstd, mv[:, 1:2], ACTF.Sqrt, bias=eps_t)
        nc.vector.reciprocal(rstd, rstd)
        out_t = work1.tile([P, D], F32, tag=tag)
        nc.vector.tensor_scalar(out_t, src, scalar1=mv[:, 0:1], scalar2=rstd,
                                op0=OP.subtract, op1=OP.mult)
        nc.vector.tensor_mul(out_t, out_t, g_b)
        nc.vector.tensor_add(out_t, out_t, be_b)
        return out_t

    def mask_prep(b):
        """Returns (ms_t [P,L] row-normalized mask, factor [P,P])."""
        mb_t = work1.tile([P, L], F32, tag="mb")
        nc.sync.dma_start(out=mb_t, in_=mask[b])
        total = work1.tile([P, 1], F32, tag="total")
        nc.vector.reduce_sum(total, mb_t, axis=AX.X, op=OP.add)
        rden = work1.tile([P, 1], F32, tag="rden")
        nc.vector.tensor_scalar_max(rden, total, 1.0)
        nc.vector.reciprocal(rden, rden)
        ms_t = work1.tile([P, L], F32, tag="ms")
        nc.vector.tensor_scalar_mul(ms_t, mb_t, rden)

        mT = transpose_chunks(mb_t, LC, "mT")      # raw mask^T  [l, c]

        # joint[c,e] = sum_l m[c,l] m[e,l]
        joint_ps = psMM.tile([P, P], F32, tag="mm")
        for ch in range(LC):
            nc.tensor.matmul(joint_ps, lhsT=mT[:, ch], rhs=mT[:, ch],
                             start=(ch == 0), stop=(ch == LC - 1))
        # broadcast total^T along rows
        totT_ps = psT.tile([1, P], F32, tag="pst")
        nc.tensor.transpose(totT_ps, total, ident)
        tot_row = work1.tile([1, P], F32, tag="totrow")
        nc.vector.tensor_copy(tot_row, totT_ps)
        totb_ps = psT.tile([P, P], F32, tag="pst")
        nc.tensor.matmul(totb_ps, lhsT=ones_row, rhs=tot_row,
                         start=True, stop=True)
        # factor = (0.5 + joint / max(total[c]+total[e], 1)) / sqrt(D)
        factor = work1.tile([P, P], F32, tag="factor")
        nc.vector.tensor_scalar_add(factor, totb_ps, total)
        nc.vector.tensor_scalar_max(factor, factor, 1.0)
        nc.vector.reciprocal(factor, factor)
        nc.vector.tensor_mul(factor, factor, joint_ps)
        nc.vector.tensor_scalar(factor, factor, 0.5, 1.0 / math.sqrt(D),
                                op0=OP.add, op1=OP.mult)
        return ms_t, factor

    def pooling(b, ms_t, inject=None):
        """Masked-mean pooling -> E_sb [P(c), D].

        E += diag(ms[:, l]) @ x[b, :, l, :], accumulated over l in PSUM.
        inject: optional {block_idx: [fn, ...]} extra issue hooks (used to
        interleave the weight DMAs into the early x stream).
        """
        psum_E = psE.tile([P, D], F32, tag="psE")
        # During the last blocks of the last batch, pad PE's idle gaps with
        # throwaway N=512 matmuls so the HAM activity monitor keeps the PE
        # clock at 2.4 GHz going into the (serial, PE-bound) final
        # transformer phase.
        warm_blocks = range(NBLK - 10, NBLK) if b == B_LOCAL - 1 else ()
        for ib, l0 in enumerate(range(0, L, NLB)):
            xt = xpool.tile([P, NLB, D], F32R, tag="xt")
            eng = nc.sync if ib % 2 == 0 else nc.scalar
            eng.dma_start(out=xt, in_=x[b, :, l0:l0 + NLB, :].bitcast(F32R))
            if inject and ib in inject:
                for fn in inject[ib]:
                    fn()
            diag = work2.tile([P, NLB, P], F32R, tag="diag")
            nc.vector.tensor_tensor(
                diag, identN,
                ms_t[:, l0:l0 + NLB, None].to_broadcast((P, NLB, P)),
                OP.mult)
            gate_mm(xt[0:1, 0, 0:1].bitcast(F32))
            for i in range(NLB):
                nc.tensor.matmul(
                    psum_E,
                    lhsT=diag[:, i],
                    rhs=xt[:, i],
                    start=(l0 == 0 and i == 0),
                    stop=(l0 == L - NLB and i == NLB - 1),
                )
            if ib in warm_blocks:
                warm_tick(3)
        if b == B_LOCAL - 1:
            warm_finish()
        E_sb = work2.tile([P, D], F32, tag="E")
        nc.vector.tensor_copy(E_sb, psum_E)
        return E_sb

    def transformer(b, E_sb, factor, w):
        wm = (b == B_LOCAL - 1)
        ET = transpose_chunks(E_sb, DC, "ET", F32R)

        # qkv projection (contract over d), bias folded in via K=1 matmul
        qkv_ps = [psMM.tile([P, D], F32, tag="mm", name=f"qkv_ps{i}")
                  for i in range(3)]
        for i in range(3):
            for j in range(DC):
                nc.tensor.matmul(qkv_ps[i], lhsT=ET[:, j],
                                 rhs=w["wqkv"][:, j, i * D:(i + 1) * D],
                                 start=(j == 0), stop=False)
            bias_mm(qkv_ps[i], w["bqkv"][:, i * D:(i + 1) * D])
        q_sb = work1.tile([P, D], F32, tag="q")
        nc.vector.tensor_copy(q_sb, qkv_ps[0])
        k_sb = work1.tile([P, D], F32, tag="k")
        nc.vector.tensor_copy(k_sb, qkv_ps[1])
        v_sb = work1.tile([P, D], F32R, tag="v")
        nc.vector.tensor_copy(v_sb, qkv_ps[2])

        # S = q k^T (fp32, N=128), then bias/scale, softmax
        qT = transpose_chunks(q_sb, DC, "qT")
        kT = transpose_chunks(k_sb, DC, "kT")
        S_ps = psMM.tile([P, P], F32, tag="mm")
        for j in range(DC):
            nc.tensor.matmul(S_ps, lhsT=qT[:, j], rhs=kT[:, j],
                             start=(j == 0), stop=(j == DC - 1))
        Sb = work1.tile([P, P], F32, tag="Sb")
        nc.vector.tensor_mul(Sb, S_ps, factor)
        nmax = work1.tile([P, 1], F32, tag="nmax")
        nc.vector.reduce_max(nmax, Sb, axis=AX.X, negate=True)
        Pexp = work1.tile([P, P], F32, tag="Pexp")
        sumexp = work1.tile([P, 1], F32, tag="sumexp")
        nc.scalar.activation(Pexp, Sb, ACTF.Exp, bias=nmax, scale=1.0,
                             accum_out=sumexp)
        rinv = work1.tile([P, 1], F32, tag="rinv")
        nc.vector.reciprocal(rinv, sumexp)
        # normalize rows of exp(S) so attn = A @ v directly
        nc.vector.tensor_scalar_mul(Pexp, Pexp, rinv)
        PT_ps = psT.tile([P, P], F32, tag="pst")
        nc.tensor.transpose(PT_ps, Pexp, ident)
        PT_sb = work1.tile([P, P], F32R, tag="PT")
        nc.vector.tensor_copy(PT_sb, PT_ps)
        attn_ps = psMM.tile([P, D], F32, tag="mm")
        nc.tensor.matmul(attn_ps, lhsT=PT_sb, rhs=v_sb, start=True, stop=True)
        attn_sb = work1.tile([P, D], F32, tag="attnsb")
        nc.vector.tensor_copy(attn_sb, attn_ps)

        # o = attn @ Wo + bo ; res1 = E + o ; E1 = LN1(res1)
        attnT = transpose_chunks(attn_sb, DC, "attnT", F32R)
        o_ps = psMM.tile([P, D], F32, tag="mm")
        for j in range(DC):
            nc.tensor.matmul(o_ps, lhsT=attnT[:, j], rhs=w["wo"][:, j],
                             start=(j == 0), stop=False)
        bias_mm(o_ps, w["bo"])
        res1 = work1.tile([P, D], F32, tag="res1")
        nc.vector.tensor_add(res1, o_ps, E_sb)
        E1 = layernorm(res1, w["g1"], w["be1"], "E1")

        # FFN: h = gelu_tanh(E1 @ W1 + b1) ; o2 = h @ W2 + b2
        E1T = transpose_chunks(E1, DC, "E1T", F32R)
        h_ps = [psMM.tile([P, D], F32, tag="mm", name=f"h_ps{i}")
                for i in range(2)]
        for i in range(2):
            for j in range(DC):
                nc.tensor.matmul(h_ps[i], lhsT=E1T[:, j],
                                 rhs=w["w1"][:, j, i * D:(i + 1) * D],
                                 start=(j == 0), stop=False)
            bias_mm(h_ps[i], w["b1"][:, i * D:(i + 1) * D])
        hpre = work1.tile([P, 2 * D], F32, tag="hpre")
        nc.vector.tensor_copy(hpre[:, 0:D], h_ps[0])
        nc.vector.tensor_copy(hpre[:, D:2 * D], h_ps[1])
        t1 = work1.tile([P, 2 * D], F32, tag="t1")
        nc.vector.tensor_mul(t1, hpre, hpre)
        nc.vector.tensor_scalar(t1, t1, 0.044715, 1.0, op0=OP.mult, op1=OP.add)
        nc.vector.tensor_mul(t1, t1, hpre)
        nc.scalar.activation(t1, t1, ACTF.Tanh,
                             scale=math.sqrt(2.0 / math.pi))
        nc.vector.tensor_scalar(t1, t1, 1.0, 0.5, op0=OP.add, op1=OP.mult)
        nc.vector.tensor_mul(t1, t1, hpre)  # t1 = gelu(hpre)
        hT = transpose_chunks(t1, D2C, "hT", F32R)
        o2_ps = psMM.tile([P, D], F32, tag="mm")
        for jj in range(D2C):
            nc.tensor.matmul(o2_ps, lhsT=hT[:, jj], rhs=w["w2"][:, jj],
                             start=(jj == 0), stop=False)
        bias_mm(o2_ps, w["b2"])
        res2 = work1.tile([P, D], F32, tag="res2")
        nc.vector.tensor_add(res2, o2_ps, E1)
        y_sb = layernorm(res2, w["g2"], w["be2"], "yln")
        nc.sync.dma_start(out=y[b], in_=y_sb)

    # ---- main schedule ----------------------------------------------------
    steps = weight_load_steps()
    # interleave the 12 weight-load DMAs into the first 24 pooling blocks
    inject = {2 + 2 * i: [fn] for i, fn in enumerate(steps)}
    for b in range(B_LOCAL):
        ms_t, factor = mask_prep(b)
        E_sb = pooling(b, ms_t, inject=inject if b == 0 else None)
        if b == 0:
            observe_weight_dmas()
        transformer(b, E_sb, factor, wstate)


def build_module() -> bass.Bass:
    # HWDGE DMA completions: one semaphore lane PER PHYSICAL RING (SP ring
    # -> lane 0, ACT ring -> lane 1). DMAs issued from one engine's ring
    # complete FIFO, so cumulative waits on that ring's lane are safe, and
    # every DMA-dependent instruction needs at most one DMA semaphore wait
    # per ring (the walrus codegen accepts only ONE sync wait per ISA
    # instruction; the NoOp spill pass below handles any leftovers).
    import concourse.tile_sem_assignment as _tsa
    _tsa.NUM_HWDGE_SEMS = 2
    if not getattr(_tsa.TileClockTick, "_ring_lane_patch", False):
        _orig_assign_tick = _tsa.TileClockTick._assign_tick

        def _assign_tick_ring_lane(self, inst):
            if isinstance(inst, _tsa.DMAInst):
                if inst.engine == mybir.EngineType.Activation:
                    self.next_hw_dma_idx = 1
                elif inst.engine == mybir.EngineType.SP:
                    self.next_hw_dma_idx = 0
            return _orig_assign_tick(self, inst)

        _tsa.TileClockTick._assign_tick = _assign_tick_ring_lane
        _tsa.TileClockTick._ring_lane_patch = True

    nc = bass.Bass()
    io = {}
    io["x"] = nc.declare_dram_parameter("x", [B_LOCAL, C, L, D], F32,
                                        isOutput=False)
    io["mask"] = nc.declare_dram_parameter("mask", [B_LOCAL, C, L], F32,
                                           isOutput=False)
    shapes = {
        "Wqkv": [D, 3 * D], "bqkv": [3 * D], "Wo": [D, D], "bo": [D],
        "W1": [D, 2 * D], "b1": [2 * D], "W2": [2 * D, D], "b2": [D],
        "g1": [D], "be1": [D], "g2": [D], "be2": [D],
    }
    for nm in WEIGHT_NAMES:
        io[nm] = nc.declare_dram_parameter(nm, shapes[nm], F32, isOutput=False)
    io["y"] = nc.declare_dram_parameter("y", [B_LOCAL, C, D], F32,
                                        isOutput=True)

    with tile.TileContext(nc) as tc:
        with ExitStack() as ctx:
            _build_kernel_body(ctx, tc, io)
    _split_multi_waits(nc)
    return nc


def _split_multi_waits(nc: bass.Bass) -> int:
    """The walrus codegen in this toolchain accepts at most ONE sync-wait
    command per ISA instruction. Tile's semaphore assignment can attach
    several. Spill all but the last wait of each instruction onto NoOp
    instructions (same engine, inserted just before it), each carrying a
    single wait — execution-equivalent since the engine stream is in-order.
    """
    import bass_rust as _br
    fn = nc.m.functions[0]
    n_spilled = 0
    for blk in fn.blocks:
        out = []
        changed = False
        for inst in blk.instructions:
            si = inst.sync_info
            if si is not None and len(si.on_wait) > 1:
                waits = list(si.on_wait)
                for w in waits[:-1]:
                    n_spilled += 1
                    nop = mybir.InstNoOp(
                        name=f"I-wspill-{n_spilled}",
                        engine=inst.engine,
                        sync_info=_br.SyncInfo(on_wait=[w], on_update=[]),
                        bass_nofuse=True,
                    )
                    nc.register_instruction(nop)
                    out.append(nop)
                inst.sync_info = _br.SyncInfo(
                    on_wait=[waits[-1]], on_update=list(si.on_update))
                changed = True
            out.append(inst)
        if changed:
            blk.instructions = out
    return n_spilled


_NC_CACHE = None


def _get_module():
    global _NC_CACHE
    if _NC_CACHE is None:
        _NC_CACHE = build_module()
    return _NC_CACHE


def kernel(**inputs) -> np.ndarray:
    arrs = {k: np.ascontiguousarray(np.asarray(v, dtype=np.float32))
            for k, v in inputs.items()}
    nc = _get_module()
    in_maps = []
    for i in range(N_CORES):
        m = {
            "x": arrs["x"][i * B_LOCAL:(i + 1) * B_LOCAL],
            "mask": arrs["mask"][i * B_LOCAL:(i + 1) * B_LOCAL],
        }
        for nm in WEIGHT_NAMES:
            m[nm] = arrs[nm]
        in_maps.append(m)
    res = run_bass_kernel_spmd(nc, in_maps, list(range(N_CORES)))
    return np.concatenate([r["y"] for r in res.results], axis=0)


if __name__ == "__main__":
    build_module()
    print("module built OK")
